# revision 32
# baseline (speedup 1.0000x reference)
"""Trainium2 Bass kernel for nn_CrossLITFusion (sparse window attention fusion).

Self-contained: hardcodes all shapes. Shards the query-pixel axis across the
8 NeuronCores (16 query rows / 8 feature rows + 3-row halo per core).

v2: host-side layout prep (block-major depth, x-major feat), transposed MLP1
(no DMA transposes), in-matmul softmax denominators, batched DMAs.
"""
import sys

sys.path.insert(0, "/opt/trn_rl_repo")

from contextlib import ExitStack

import ml_dtypes
import numpy as np

import concourse.bass as bass
import concourse.bacc as bacc
import concourse.mybir as mybir
import concourse.tile as tile

BF = ml_dtypes.bfloat16
dt = mybir.dt
AF = mybir.ActivationFunctionType
ALU = mybir.AluOpType

# Problem constants
DIM = 64
HEAD = 2
DH = 32
R = 3
HF, WF = 64, 64
HQ, WQ = 128, 128
HID = 256
N_CORES = 8

QROWS = 16             # query rows per core
FROWS = 8              # feature rows owned
PROWS = FROWS + 2 * R  # 14 padded feature rows (halo)
PW = WF + 2 * R        # 70 padded feature cols
NBLK = WF // 2         # 32 col-blocks (4 query cols each)
BKP = 8 * PROWS        # 112 kpix per block window
NSB = 8                # superblocks (4 blocks each)
QCORE = QROWS * WQ     # 2048 queries per core

# blobA: kv-conv weights (needed first); blobB: the rest
A_WKVP = 0      # [128, 3, 128]
A_WKV2 = 384    # [64:128, 3, 128]
A_ID = 768      # [64:128, 64]
A_MSK = 832     # [128, 32*14]
NBA = 1280
B_WQP = 0       # [128, 3, 64]
B_WQ2 = 192     # [64:128, 3, 64]
B_W1 = 384      # [0:65, 256]
B_W2 = 640      # [128, 2, 64]
B_EXP = 768     # [0:112, 512]
NBB = 1280

_CACHE = {}

# attention pipeline orders queries block-major: qa = 64*b + 4*qy + qxl
# (b = feat col-block = qx//4). _QPERM[qa] = row-major index 128*qy + 4*b + qxl.
_QA = np.arange(QCORE)
_QPERM = 128 * ((_QA % 64) // 4) + 4 * (_QA // 64) + (_QA % 4)


def _build_program():
    nc = bacc.Bacc("TRN2", target_bir_lowering=False, debug=False)

    # ---- DRAM I/O ----
    blobA_d = nc.dram_tensor("blobA", [128, NBA], dt.bfloat16, kind="ExternalInput").ap()
    blobB_d = nc.dram_tensor("blobB", [128, NBB], dt.bfloat16, kind="ExternalInput").ap()
    blob32_d = nc.dram_tensor("blob32", [128, 2], dt.float32, kind="ExternalInput").ap()
    d2_d = nc.dram_tensor("d2blk", [128, NBLK, 17, 6], dt.bfloat16, kind="ExternalInput").ap()
    f2_d = nc.dram_tensor("f2d", [128, PW, 16], dt.bfloat16, kind="ExternalInput").ap()
    dres_d = nc.dram_tensor("dres3", [64, QCORE], dt.float32, kind="ExternalInput").ap()
    out_d = nc.dram_tensor("out", [64, QCORE], dt.float32, kind="ExternalOutput").ap()

    with tile.TileContext(nc) as tc, ExitStack() as ctx:
        const = ctx.enter_context(tc.tile_pool(name="const", bufs=1))
        maps = ctx.enter_context(tc.tile_pool(name="maps", bufs=1))

        # ---- SBUF tiles ----
        cbA = const.tile([128, NBA], dt.bfloat16)
        cbB = const.tile([128, NBB], dt.bfloat16)
        cb32 = const.tile([128, 2], dt.float32)
        d2 = maps.tile([128, NBLK, 17, 6], dt.bfloat16)
        f2 = maps.tile([128, PW, 16], dt.bfloat16)
        dresT = maps.tile([64, QCORE], dt.float32)
        qmap = maps.tile([64, NBLK, 64], dt.bfloat16)
        q2 = maps.tile([64, NBLK, 128], dt.bfloat16)
        kv = maps.tile([128, PW, PROWS], dt.bfloat16)
        vt = maps.tile([BKP, NBLK, 96], dt.bfloat16)
        attn = maps.tile([65, QCORE], dt.bfloat16)
        hg = maps.tile([128, 2, QCORE], dt.bfloat16)
        oT = maps.tile([64, QCORE], dt.float32)

        # ---- input DMAs: spread across queues; blobA/f2 gate the first convs ----
        nc.sync.dma_start(cbA[:], blobA_d)
        nc.scalar.dma_start(f2[:], f2_d)
        nc.sync.dma_start(cbB[:], blobB_d)
        nc.sync.dma_start(cb32[:], blob32_d)
        nc.sync.dma_start(d2[:], d2_d)
        nc.gpsimd.dma_start(dresT[:], dres_d)

        # ---- const views ----
        wqp = cbB[:, B_WQP : B_WQP + 192].rearrange("c (k m) -> c k m", k=3)
        wq2 = cbB[64:128, B_WQ2 : B_WQ2 + 192].rearrange("c (k m) -> c k m", k=3)
        wkvp = cbA[:, A_WKVP : A_WKVP + 384].rearrange("c (k m) -> c k m", k=3)
        wkv2 = cbA[64:128, A_WKV2 : A_WKV2 + 384].rearrange("c (k m) -> c k m", k=3)
        ident = cbA[64:128, A_ID : A_ID + 64]
        w1a = cbB[0:65, B_W1 : B_W1 + 256]
        w2 = cbB[:, B_W2 : B_W2 + 128].rearrange("c (g m) -> c g m", g=2)
        msk = cbA[:, A_MSK : A_MSK + 448].rearrange("c (x y) -> c x y", x=32)
        expt = cbB[0:112, B_EXP : B_EXP + 512]
        bq = cb32[0:64, 0:1]
        bkv = cb32[:, 1:2]

        # ---- memsets (off critical path) ----
        nc.gpsimd.memset(q2[0:32, :, 64:128], 0.0)
        nc.gpsimd.memset(q2[32:64, :, 0:64], 0.0)
        nc.vector.memset(kv[:, 0:R, :], 0.0)
        nc.vector.memset(kv[:, R + WF : PW, :], 0.0)
        nc.gpsimd.memset(vt[:, :, 64:96], 1.0)
        nc.vector.memset(attn[64:65, :], 1.0)

        # ---- conv_k + conv_v fused -> KV x-major (128, 70, 14) ----
        # kv[c, x, y]: K channels rows 0-63, V rows 64-127
        with tc.tile_pool(name="kv_ps", bufs=2, space="PSUM") as kv_ps:
            for xc in range(2):
                x0 = R + 32 * xc
                acc = kv_ps.tile([128, 32, PROWS], dt.float32, tag="kv")
                for ky in range(3):
                    nc.tensor.matmul(acc[:], wkvp[:, ky, :],
                                     f2[:, x0 - 1 : x0 + 31, ky : ky + PROWS],
                                     start=(ky == 0), stop=False)
                for ky in range(3):
                    nc.tensor.matmul(acc[:], wkv2[:, ky, :],
                                     f2[64:128, x0 : x0 + 32, ky : ky + PROWS],
                                     start=False, stop=(ky == 2))
                # (conv + bias) * row-validity mask
                nc.vector.scalar_tensor_tensor(
                    kv[0:64, x0 : x0 + 32, :], acc[0:64], bkv[0:64],
                    msk[0:64], op0=ALU.add, op1=ALU.mult)
                nc.vector.scalar_tensor_tensor(
                    kv[64:128, x0 : x0 + 32, :], acc[64:128], bkv[64:128],
                    msk[64:128], op0=ALU.add, op1=ALU.mult)

        # ---- V_T tiles (112, 96) per block: [Vh0(32) | Vh1(32) | ones(32)] ----
        with tc.tile_pool(name="vt_ps", bufs=3, space="PSUM") as vt_ps:
            for b in range(NBLK):
                tp = vt_ps.tile([BKP, 64], dt.bfloat16, tag="vt")
                nc.tensor.transpose(tp[:], kv[64:128, 2 * b : 2 * b + 8, :], ident)
                nc.scalar.copy(vt[:, b, 0:64], tp[:])

        # ---- phase 1: conv_q chunks + attention ----
        qsc = float(1.0 / np.sqrt(DH))
        with tc.tile_pool(name="cq_ps", bufs=1, space="PSUM") as cq_ps, \
             tc.tile_pool(name="sc_ps", bufs=3, space="PSUM") as sc_ps, \
             tc.tile_pool(name="p_pool", bufs=2) as p_pool:

            def conv_q_chunk(c):
                b0 = 8 * c
                acc = cq_ps.tile([64, 512], dt.float32, tag="cq")
                accv = acc[:].rearrange("c (b y x) -> c b y x", b=8, y=16)
                for kx in range(3):
                    nc.tensor.matmul(accv, wqp[:, kx, :],
                                     d2[:, b0 : b0 + 8, 0:16, kx : kx + 4],
                                     start=(kx == 0), stop=False)
                for kx in range(3):
                    nc.tensor.matmul(accv, wq2[:, kx, :],
                                     d2[64:128, b0 : b0 + 8, 1:17, kx : kx + 4],
                                     start=False, stop=(kx == 2))
                nc.vector.tensor_scalar(qmap[:, b0 : b0 + 8, :], accv,
                                        qsc, bq, op0=ALU.mult, op1=ALU.add)
                nc.sync.dma_start(q2[0:32, b0 : b0 + 8, 0:64],
                                  qmap[0:32, b0 : b0 + 8, :])
                nc.sync.dma_start(q2[32:64, b0 : b0 + 8, 64:128],
                                    qmap[32:64, b0 : b0 + 8, :])

            def attention_sb(sb):
                qkp = sc_ps.tile([128, 512], dt.float32, tag="sc")
                for j in range(4):
                    b = 4 * sb + j
                    nc.tensor.matmul(qkp[0:112, 128 * j : 128 * j + 128],
                                     kv[0:64, 2 * b : 2 * b + 8, :], q2[:, b, :],
                                     start=True, stop=True)
                pex = p_pool.tile([BKP, 512], dt.bfloat16, tag="pex")
                nc.scalar.activation(pex[:], qkp[0:112, :], AF.Exp)
                pw = p_pool.tile([BKP, 512], dt.bfloat16, tag="pw")
                nc.vector.tensor_mul(pw[:], pex[:], expt)
                avp = sc_ps.tile([128, 4, 128], dt.float32, tag="sc")
                for j in range(4):
                    b = 4 * sb + j
                    nc.tensor.matmul(avp[0:96, j, :], vt[:, b, :],
                                     pw[:, 128 * j : 128 * j + 128],
                                     start=True, stop=True)
                a0 = attn[0:32, 256 * sb : 256 * sb + 256].rearrange(
                    "p (j c) -> p j c", j=4)
                a1 = attn[32:64, 256 * sb : 256 * sb + 256].rearrange(
                    "p (j c) -> p j c", j=4)
                den = p_pool.tile([64, 4, 64], dt.float32, tag="den")
                nc.scalar.copy(den[0:32], avp[64:96, :, 0:64])
                nc.scalar.copy(den[32:64], avp[64:96, :, 64:128])
                rcp = p_pool.tile([64, 4, 64], dt.float32, tag="rcp")
                nc.vector.reciprocal_approx_fast(rcp[:], den[:])
                nc.vector.tensor_mul(a0, avp[0:32, :, 0:64], rcp[0:32])
                nc.vector.tensor_mul(a1, avp[32:64, :, 64:128], rcp[32:64])

            for c in range(4):
                conv_q_chunk(c)
                attention_sb(2 * c)
                attention_sb(2 * c + 1)

        # ---- phase 2: MLP; wait_until batches the Gelus after the Exps so the
        # scalar engine loads each activation table once ----
        with tc.tile_pool(name="m1_ps", bufs=2, space="PSUM") as m1_ps, \
             tc.tile_pool(name="m2_ps", bufs=1, space="PSUM") as m2_ps, \
             tc.tile_wait_until(0.042):
            for c in range(2):
                for g in range(2):
                    m1 = m1_ps.tile([128, 1024], dt.float32, tag="m1")
                    for h in range(2):
                        nc.tensor.matmul(m1[:, 512 * h : 512 * h + 512],
                                         w1a[:, 128 * g : 128 * g + 128],
                                         attn[:, 1024 * c + 512 * h : 1024 * c + 512 * h + 512],
                                         start=True, stop=True)
                    nc.scalar.activation(hg[:, g, 1024 * c : 1024 * c + 1024],
                                         m1[:], AF.Gelu)
                for h in range(2):
                    c2 = 2 * c + h
                    m2 = m2_ps.tile([64, 512], dt.float32, tag="m2")
                    nc.tensor.matmul(m2[:], w2[:, 0, :],
                                     hg[:, 0, 512 * c2 : 512 * c2 + 512],
                                     start=True, stop=False)
                    nc.tensor.matmul(m2[:], w2[:, 1, :],
                                     hg[:, 1, 512 * c2 : 512 * c2 + 512],
                                     start=False, stop=True)
                    nc.vector.tensor_add(oT[:, 512 * c2 : 512 * c2 + 512], m2[:],
                                         dresT[:, 512 * c2 : 512 * c2 + 512])

        nc.sync.dma_start(out_d, oT[:])

    nc.compile()
    return nc


def _host_prep(depth, x, cell, conv_q_w, conv_q_b, conv_k_w, conv_k_b,
               conv_v_w, conv_v_b, cpb_w1, cpb_b1, cpb_w2,
               mlp_w1, mlp_b1, mlp_w2, mlp_b2):
    """Build the 8 per-core input maps."""
    f32 = np.float32
    depth = np.asarray(depth, f32)
    x = np.asarray(x, f32)
    cell = np.asarray(cell, f32)

    depth_T = np.ascontiguousarray(depth[0].T).reshape(64, HQ, WQ)
    feat_T = np.ascontiguousarray(x[0].T).reshape(64, HF, WF)

    wq = np.asarray(conv_q_w, f32)
    wk = np.asarray(conv_k_w, f32)
    wv = np.asarray(conv_v_w, f32)

    blobA = np.zeros((128, NBA), f32)
    blobB = np.zeros((128, NBB), f32)
    # wq pair: rows c -> ky=0, rows 64+c -> ky=1, per kx
    for kx in range(3):
        blobB[0:64, B_WQP + 64 * kx : B_WQP + 64 * kx + 64] = wq[:, :, 0, kx].T
        blobB[64:128, B_WQP + 64 * kx : B_WQP + 64 * kx + 64] = wq[:, :, 1, kx].T
        blobB[64:128, B_WQ2 + 64 * kx : B_WQ2 + 64 * kx + 64] = wq[:, :, 2, kx].T
    # wkv pair: rows c -> kx=0, rows 64+c -> kx=1, per ky; cols [K(64) | V(64)]
    for ky in range(3):
        o = A_WKVP + 128 * ky
        blobA[0:64, o : o + 64] = wk[:, :, ky, 0].T
        blobA[0:64, o + 64 : o + 128] = wv[:, :, ky, 0].T
        blobA[64:128, o : o + 64] = wk[:, :, ky, 1].T
        blobA[64:128, o + 64 : o + 128] = wv[:, :, ky, 1].T
        o = A_WKV2 + 128 * ky
        blobA[64:128, o : o + 64] = wk[:, :, ky, 2].T
        blobA[64:128, o + 64 : o + 128] = wv[:, :, ky, 2].T
    blobA[64:128, A_ID : A_ID + 64] = np.eye(64, dtype=f32)

    # MLP weights
    m1w = np.asarray(mlp_w1, f32)
    m1b = np.asarray(mlp_b1, f32)
    m2w = np.asarray(mlp_w2, f32)
    m2b = np.asarray(mlp_b2, f32)
    rel_cell = cell[0] * np.array([HF, WF], f32)
    b1pp = m1b + m1w[:, 64:66] @ rel_cell
    blobB[0:32, B_W1 : B_W1 + 256] = m1w[:, 0:32].T
    blobB[32:64, B_W1 : B_W1 + 256] = m1w[:, 32:64].T
    blobB[64, B_W1 : B_W1 + 256] = b1pp
    blobB[:, B_W2 : B_W2 + 64] = m2w[:, 0:128].T
    blobB[:, B_W2 + 64 : B_W2 + 128] = m2w[:, 128:256].T

    # position-bias table -> multiplicative exp table with window mask
    w1 = np.asarray(cpb_w1, f32)
    b1 = np.asarray(cpb_b1, f32)
    w2c = np.asarray(cpb_w2, f32)
    dy = (np.linspace(-R, R, 2 * R + 1).astype(f32)) * f32(2.0 / HF)
    delta = np.stack(np.meshgrid(dy, dy, indexing="ij"), -1).reshape(-1, 2)
    pb = np.zeros((HEAD, 2, 2, 7, 7), f32)
    for iy in range(2):
        for jx in range(2):
            sy = f32(-1.0) + (2 * iy + 1) / f32(HQ)
            sx = f32(-1.0) + (2 * jx + 1) / f32(WQ)
            base = np.array([f32(-1.0) + 1.0 / f32(HF), f32(-1.0) + 1.0 / f32(WF)], f32)
            ck = base[None, :] + delta
            rel = (np.array([sy, sx], f32)[None, :] - ck) * np.array([HQ, WQ], f32)
            h = np.maximum(rel @ w1.T + b1, 0.0)
            p = h @ w2c.T
            pb[:, iy, jx] = p.T.reshape(HEAD, 7, 7)

    exptab = np.zeros((BKP, 128), f32)
    for kp in range(BKP):
        xk, py = kp // PROWS, kp % PROWS
        for n in range(128):
            h, r = n // 64, n % 64
            qy, qx = r // 4, r % 4
            dyy = py - (qy // 2 + R)
            dxx = xk - (qx // 2 + R)
            if abs(dyy) <= R and abs(dxx) <= R:
                exptab[kp, n] = np.exp(pb[h, qy % 2, qx % 2, dyy + R, dxx + R])
    blobB[0:BKP, B_EXP : B_EXP + 512] = np.tile(exptab, (1, 4))

    blob32 = np.zeros((128, 2), f32)
    blob32[0:64, 0] = np.asarray(conv_q_b, f32) / np.sqrt(f32(DH))
    blob32[0:64, 1] = np.asarray(conv_k_b, f32)
    blob32[64:128, 1] = np.asarray(conv_v_b, f32)

    # padded global maps
    dpad = np.zeros((64, HQ + 2, WQ + 2), f32)
    dpad[:, 1 : 1 + HQ, 1 : 1 + WQ] = depth_T
    fpad = np.zeros((64, HF + 9, HF + 7), f32)
    fpad[:, 4 : 4 + HF, 3 : 3 + WF] = feat_T
    bidx = np.arange(NBLK)[:, None] * 4 + np.arange(6)[None, :]  # (32, 6)

    in_maps = []
    blobA = blobA.astype(BF)
    blobB16 = np.ascontiguousarray(blobB.astype(BF))
    for t in range(N_CORES):
        base = dpad[:, 16 * t : 16 * t + 18, :]          # (64, 18, 130)
        d2a = base[:, 0:17, :][:, :, bidx]               # (64, 17, 32, 6)
        d2b = base[:, 1:18, :][:, :, bidx]
        d2blk = np.concatenate([d2a.transpose(0, 2, 1, 3),
                                d2b.transpose(0, 2, 1, 3)], 0)
        f2a = fpad[:, 8 * t : 8 * t + 16, 0:PW].transpose(0, 2, 1)  # (64, 70, 16)
        f2b = fpad[:, 8 * t : 8 * t + 16, 1 : 1 + PW].transpose(0, 2, 1)
        f2d = np.concatenate([f2a, f2b], 0)

        mrow = np.zeros((PROWS,), f32)
        for py in range(PROWS):
            if 0 <= FROWS * t - R + py < HF:
                mrow[py] = 1.0
        bA = blobA.copy()
        bA[:, A_MSK : A_MSK + 448] = np.tile(mrow, (128, 32)).astype(BF)

        dres_rows = depth[0, QCORE * t : QCORE * (t + 1), :] + m2b[None, :]
        dres3 = dres_rows[_QPERM].T

        m = dict(blobA=np.ascontiguousarray(bA),
                 blobB=blobB16,
                 blob32=blob32,
                 d2blk=np.ascontiguousarray(d2blk.astype(BF)),
                 f2d=np.ascontiguousarray(f2d.astype(BF)),
                 dres3=np.ascontiguousarray(dres3.astype(f32)))
        in_maps.append(m)
    return in_maps


LAST_RESULT = None


def _prep_inputs(inputs):
    keys = dict(inputs)
    keys.pop("shape_y", None)
    keys.pop("shape_x", None)
    return _host_prep(**keys)


def kernel(**inputs):
    global LAST_RESULT
    from concourse.bass_utils import run_bass_kernel_spmd

    if "nc" not in _CACHE:
        _CACHE["nc"] = _build_program()
    nc = _CACHE["nc"]
    in_maps = _prep_inputs(inputs)
    res = run_bass_kernel_spmd(nc, in_maps, core_ids=list(range(N_CORES)))
    LAST_RESULT = res
    parts = []
    for t in range(N_CORES):
        oc = np.ascontiguousarray(np.asarray(res.results[t]["out"]).reshape(64, QCORE).T)
        orow = np.empty_like(oc)
        orow[_QPERM] = oc
        parts.append(orow)
    out = np.concatenate(parts, 0)
    return out[None].astype(np.float32)


def _patch_sim_gelu():
    import math
    import concourse.bass_interp as bi
    import concourse.mybir as mb
    if getattr(bi.InstructionExecutor, "_gelu_patched", False):
        return
    orig = bi.InstructionExecutor.visit_InstActivation
    from concourse.bass_interp import Direction

    erf = np.vectorize(math.erf)

    def patched(self, instruction, *, reg_snapshot=None):
        if getattr(instruction, "func", None) == mb.ActivationFunctionType.Gelu:
            instruction.func = mb.ActivationFunctionType.Identity
            try:
                r = orig(self, instruction, reg_snapshot=reg_snapshot)
            finally:
                instruction.func = mb.ActivationFunctionType.Gelu
            ov = self.view_ap(instruction.outs[0], Direction.WRITE, instruction,
                              reg_snapshot=reg_snapshot)
            x = np.asarray(ov[:], dtype=np.float64)
            ov[:] = (0.5 * x * (1.0 + erf(x / np.sqrt(2.0)))).astype(np.float32)
            return r
        return orig(self, instruction, reg_snapshot=reg_snapshot)

    bi.InstructionExecutor.visit_InstActivation = patched
    bi.InstructionExecutor._gelu_patched = True


def simulate_core(core=0, inputs=None, dbg=False):
    """CoreSim single-core check helper (dev only)."""
    from concourse.bass_interp import CoreSim

    _patch_sim_gelu()

    if "nc" not in _CACHE:
        _CACHE["nc"] = _build_program()
    nc = _CACHE["nc"]
    in_maps = _prep_inputs(inputs)
    sim = CoreSim(nc, trace=False)
    for k, v in in_maps[core].items():
        sim.tensor(k)[:] = v
    sim.simulate(check_with_hw=False)
    return np.ascontiguousarray(np.array(sim.tensor("out")).reshape(64, QCORE).T)
